# revision 1
# baseline (speedup 1.0000x reference)
"""Discounted cumsum (y[b,h,t,d] = x[b,h,t,d] + gamma[h] * y[b,h,t-1,d]) on 8 trn2 cores.

Blocked parallel scan, pure data parallelism over the B*H=64 (b,h) pairs (8 per core).
SBUF layout per pair: [128 part = t-within-block, 32 blocks x 128 d]; the within-block
scan, the per-block sums, and the carry injection are all PE matmuls batched 4 blocks
(N=512 moving columns) per instruction; the 32 block carries come from one small
matmul with the geometric-decay matrix.

Precision/speed: the matmul moving operand is split hi/lo into two 16-bit tensors
(host-side), so each logical matmul is 2-3 accumulating PE matmuls at full bf16/fp16
rate. Two per-slot schemes share the program:
  - large gamma (>= 0.55): change of variable x'_s = gamma^-s x_s makes the scan
    coefficients a triangular ONES matrix - exactly representable in bf16, so the
    only error is the ~2^-16 hi/lo residual. The output is rescaled by gamma^t via
    the copy-out's per-partition scalar. Requires gamma^-127 to stay in fp32 range.
  - small gamma: fp16 gamma-power coefficients (2^-11) with a third scan matmul
    (lo-coefficients x hi-data) pushing the scan error to ~2^-22.
Pairs are re-assigned to cores so that each program slot p holds the same scheme on
every core (SPMD: one program). Host precomputes all gamma-power constants in float64
and pre-transposes the hi/lo inputs so input DMAs are contiguous 8KB lines.

Walrus allows 1 sync wait on engine instructions / 2 on DMAs; after Tile scheduling,
bass_rust.generate_event_semaphores legalizes by moving excess waits onto
InstEventSemaphore carriers. The tiny bf16 ldweights "absorbers" advance PE's
observed DMA-lane clocks early so hot-path matmuls need at most their one wait.
"""

import numpy as np

B, H, S, D = 4, 16, 4096, 128
T = 128          # block length (matmul contraction dim)
KB = S // T      # 32 blocks per sequence
NG = 4           # blocks per matmul group (4*128 = 512 moving columns)
G = KB // NG     # 8 groups per pair
NCORES = 8
PAIRS = (B * H) // NCORES  # 8 pair-slots per core
GAMMA_ONES_MIN = 0.55      # scaled scheme needs gamma^-127 * |x'| well inside fp32

_nc_cache = {}


def _build_program(slot_large):
    """slot_large: tuple of PAIRS bools - per-slot scheme, identical on all cores."""
    key = tuple(slot_large)
    if key in _nc_cache:
        return _nc_cache[key]

    import concourse.bass as bass
    import concourse.mybir as mybir
    from concourse.tile import TileContext

    f32 = mybir.dt.float32
    bf16 = mybir.dt.bfloat16
    fp16 = mybir.dt.float16

    nc = bass.Bass(trn_type="TRN2")

    # 16-bit tensors are declared bf16; small-gamma slots bitcast slices to fp16.
    xh_d = nc.declare_dram_parameter("x_hi", [PAIRS, T, KB * D], bf16, isOutput=False)
    xl_d = nc.declare_dram_parameter("x_lo", [PAIRS, T, KB * D], bf16, isOutput=False)
    A_d = nc.declare_dram_parameter("A_all", [T, PAIRS * T], bf16, isOutput=False)
    u_d = nc.declare_dram_parameter("u_all", [T, PAIRS], bf16, isOutput=False)
    g_d = nc.declare_dram_parameter("g_all", [1, PAIRS * T], bf16, isOutput=False)
    GT_d = nc.declare_dram_parameter("GT_all", [KB, PAIRS * KB], f32, isOutput=False)
    scl_d = nc.declare_dram_parameter("scl_all", [T, PAIRS], f32, isOutput=False)
    y_d = nc.declare_dram_parameter("y", [PAIRS, S, D], f32, isOutput=True)

    def sl16(ap, p):
        # per-slot element dtype for 16-bit constants/data
        return ap if slot_large[p] else ap.bitcast(fp16)

    with TileContext(nc) as tc:
        with (
            tc.tile_pool(name="const", bufs=1) as cpool,
            tc.tile_pool(name="xin", bufs=4) as xpool,
            tc.tile_pool(name="yout", bufs=2) as ypool,
            tc.tile_pool(name="rfl", bufs=2) as rfpool,
            tc.tile_pool(name="r32", bufs=8) as r32pool,
            tc.tile_pool(name="c32", bufs=8) as c32pool,
            tc.tile_pool(name="cfl", bufs=4) as cfpool,
            tc.tile_pool(name="grp_ps", bufs=4, space="PSUM") as gp_pool,
            tc.tile_pool(name="mmr_ps", bufs=2, space="PSUM") as rp_pool,
            tc.tile_pool(name="c_ps", bufs=2, space="PSUM") as cp_pool,
        ):
            Ac = cpool.tile([T, PAIRS * T], bf16, tag="Ac")
            uc = cpool.tile([T, PAIRS], bf16, tag="uc")
            gc = cpool.tile([1, PAIRS * T], bf16, tag="gc")
            GTc = cpool.tile([KB, PAIRS * KB], f32, tag="GTc")
            sclc = cpool.tile([T, PAIRS], f32, tag="sclc")
            nc.gpsimd.dma_start(out=Ac[:], in_=A_d[:])
            nc.gpsimd.dma_start(out=uc[:], in_=u_d[:])
            nc.gpsimd.dma_start(out=gc[:], in_=g_d[:])
            nc.gpsimd.dma_start(out=GTc[:], in_=GT_d[:])
            nc.gpsimd.dma_start(out=sclc[:], in_=scl_d[:])

            def absorb(ap_src):
                # standalone bf16 ldweights: makes PE wait on that tile's DMA
                # lane here; the real matmuls self-load their own stationary.
                nc.tensor.ldweights(ap_src.bitcast(bf16))

            absorb(Ac[0:1, 0:1])
            absorb(uc[0:1, 0:1])
            absorb(gc[0:1, 0:1])
            absorb(GTc[0:1, 0:1].bitcast(bf16))
            absorb(sclc[0:1, 0:1].bitcast(bf16))

            for p in range(PAIRS):
                large = slot_large[p]
                # ---- load pair (hi/lo pre-transposed on host: contiguous rows)
                Xh = xpool.tile([T, KB * D], bf16, tag="Xh")
                nc.sync.dma_start(out=Xh[:], in_=xh_d[p])
                Xl = xpool.tile([T, KB * D], bf16, tag="Xl")
                nc.sync.dma_start(out=Xl[:], in_=xl_d[p])
                absorb(Xh[0:1, 0:1])
                absorb(Xl[0:1, 0:1])

                # ---- block sums r'_k (scaled space for large slots)
                Rflat = rfpool.tile([1, KB * D], f32, tag="Rflat")
                for g in range(G):
                    sl = slice(g * NG * D, (g + 1) * NG * D)
                    rp = rp_pool.tile([1, NG * D], f32, tag="rp")
                    nc.tensor.matmul(
                        rp[:], lhsT=sl16(uc[:, p : p + 1], p),
                        rhs=sl16(Xh[:, sl], p), start=True, stop=not large,
                    )
                    if large:
                        nc.tensor.matmul(
                            rp[:], lhsT=sl16(uc[:, p : p + 1], p),
                            rhs=sl16(Xl[:, sl], p), start=False, stop=True,
                        )
                    if g < 4:
                        nc.vector.tensor_copy(out=Rflat[:, sl], in_=rp[:])
                    else:
                        nc.scalar.copy(out=Rflat[:, sl], in_=rp[:])
                # scatter [1,(k d)] -> [KB part, d] on the SP ring (the ACT
                # ring carries the big out-DMAs whose descriptor generation
                # would delay this chain-critical transfer). Flat orders zip.
                R32 = r32pool.tile([KB, D], f32, tag="R32")
                nc.sync.dma_start(out=R32[:], in_=Rflat[:])

                # ---- carries: C[k] = carry into block k (times gamma, scaled,
                # for large slots - folded into GT host-side)
                cp = cp_pool.tile([KB, D], f32, tag="cp")
                nc.tensor.matmul(
                    cp[:], lhsT=GTc[:, p * KB : (p + 1) * KB], rhs=R32[:],
                    start=True, stop=True,
                )
                C32h = c32pool.tile([KB, D], bf16, tag="C32h")
                nc.vector.tensor_copy(out=sl16(C32h[:], p), in_=cp[:])
                cfh = cfpool.tile([1, KB * D], bf16, tag="cfh")
                nc.sync.dma_start(out=cfh[:], in_=C32h[:])
                absorb(cfh[0:1, 0:1])
                if large:
                    C32l = c32pool.tile([KB, D], bf16, tag="C32l")
                    nc.vector.tensor_tensor(
                        out=C32l[:], in0=cp[:], in1=C32h[:],
                        op=mybir.AluOpType.subtract,
                    )
                    cfl = cfpool.tile([1, KB * D], bf16, tag="cfl")
                    nc.sync.dma_start(out=cfl[:], in_=C32l[:])
                    absorb(cfl[0:1, 0:1])

                # ---- per group: carry injection, then within-block scan; the
                # copy-out applies the per-partition unscale factor.
                Ys = ypool.tile([T, KB * D], f32, tag="Ys")
                for g in range(G):
                    grp = gp_pool.tile([T, NG * D], f32, tag="grp")
                    sl = slice(g * NG * D, (g + 1) * NG * D)
                    gv = sl16(gc[:, p * T : (p + 1) * T], p)
                    nc.tensor.matmul(
                        grp[:], lhsT=gv, rhs=sl16(cfh[:, sl], p),
                        start=True, stop=False,
                    )
                    if large:
                        nc.tensor.matmul(
                            grp[:], lhsT=gv, rhs=sl16(cfl[:, sl], p),
                            start=False, stop=False,
                        )
                    Ap = sl16(Ac[:, p * T : (p + 1) * T], p)
                    nc.tensor.matmul(
                        grp[:], lhsT=Ap, rhs=sl16(Xh[:, sl], p),
                        start=False, stop=False,
                    )
                    nc.tensor.matmul(
                        grp[:], lhsT=Ap, rhs=sl16(Xl[:, sl], p),
                        start=False, stop=True,
                    )
                    nc.vector.tensor_scalar_mul(
                        out=Ys[:, sl], in0=grp[:], scalar1=sclc[:, p : p + 1]
                    )

                # ---- store pair
                nc.scalar.dma_start(
                    out=y_d[p].rearrange("(k s) d -> s k d", s=T),
                    in_=Ys[:].rearrange("s (k d) -> s k d", k=KB),
                )

    # Split excess per-instruction sync waits onto InstEventSemaphore carriers.
    import bass_rust

    bass_rust.generate_event_semaphores(nc)

    _nc_cache[key] = nc
    return nc


def _pair_assignment(gam):
    """Assign the 64 (b,h) pairs to (core, slot) so each slot's scheme is
    core-uniform. Returns (order, slot_large): order[c*PAIRS+p] = global pair id
    (b*H+h) placed at core c, slot p."""
    large_heads = [h for h in range(H) if gam[h] >= GAMMA_ONES_MIN]
    small_heads = [h for h in range(H) if gam[h] < GAMMA_ONES_MIN]
    large_pairs = [b * H + h for h in large_heads for b in range(B)]
    small_pairs = [b * H + h for h in small_heads for b in range(B)]
    n_large_slots = len(large_pairs) // NCORES  # leftovers run as "small" (fp16)
    # shortest chain (a small slot) first shrinks the pipeline-fill stall
    slot_large = [False] + [True] * n_large_slots + [False] * (
        PAIRS - n_large_slots - 1
    )
    ordered = (
        small_pairs[: NCORES]
        + large_pairs
        + small_pairs[NCORES:]
        + large_pairs[NCORES * n_large_slots :]
    )
    # slot s across cores c takes ordered[s*NCORES + c]
    order = [0] * (NCORES * PAIRS)
    for s in range(PAIRS):
        for c in range(NCORES):
            order[c * PAIRS + s] = ordered[s * NCORES + c]
    return order, tuple(slot_large)


def _host_constants(g, large):
    """Per-pair constants from float64 gamma powers."""
    pw = np.power(g, np.arange(2 * S, dtype=np.float64))
    t_idx = np.arange(T)
    if large:
        A = np.triu(np.ones((T, T)))  # [s, t]: ones for t >= s (exact in bf16)
        A2 = np.zeros((T, T))
        u = np.ones(T)
        gv = np.ones(T)
        scl = pw[t_idx]  # y_t = gamma^t * y'_t
        gt_extra = pw[127] * g  # r = gamma^127 r' ; carry coefficient gamma^(t+1)
        xscale = np.power(g, -t_idx.astype(np.float64))
    else:
        t_minus_s = t_idx[None, :] - t_idx[:, None]
        A = np.where(t_minus_s >= 0, pw[np.clip(t_minus_s, 0, None)], 0.0)
        A2 = None  # fp16 lo of A, filled at pack time
        u = pw[127 - t_idx]
        gv = pw[t_idx + 1]
        scl = np.ones(T)
        gt_extra = 1.0
        xscale = None
    pw128 = np.power(pw[T], np.arange(KB, dtype=np.float64))
    k_minus_j = np.arange(KB)[None, :] - 1 - np.arange(KB)[:, None]
    GT = np.where(k_minus_j >= 0, pw128[np.clip(k_minus_j, 0, None)], 0.0) * gt_extra
    return A, A2, u, gv, GT, scl, xscale


def _make_in_maps(tensor, gamma):
    import ml_dtypes

    bf16 = ml_dtypes.bfloat16
    x = np.asarray(tensor, dtype=np.float32).reshape(B * H, S, D)
    gam = np.asarray(gamma, dtype=np.float64).reshape(H)
    order, slot_large = _pair_assignment(gam)

    in_maps = []
    for c in range(NCORES):
        xh = np.empty((PAIRS, T, KB * D), bf16)
        xl = np.empty((PAIRS, T, KB * D), bf16)
        A_all = np.zeros((T, PAIRS * T), bf16)
        u_all = np.zeros((T, PAIRS), bf16)
        g_all = np.zeros((1, PAIRS * T), bf16)
        GT_all = np.zeros((KB, PAIRS * KB), np.float32)
        scl_all = np.zeros((T, PAIRS), np.float32)
        for p in range(PAIRS):
            pid = order[c * PAIRS + p]
            g = gam[pid % H]
            large = slot_large[p]
            A, A2, u, gv, GT, scl, xscale = _host_constants(g, large)
            # x in scan layout [s, (k, d)]
            xp = x[pid].reshape(KB, T, D).transpose(1, 0, 2).reshape(T, KB * D)
            xp = xp.astype(np.float64)
            if large:
                xp = xp * xscale[:, None]
                hi = xp.astype(bf16)
                lo = (xp - hi.astype(np.float64)).astype(bf16)
                A_all[:, p * T : (p + 1) * T] = A.astype(bf16)
                u_all[:, p] = u.astype(bf16)
                g_all[0, p * T : (p + 1) * T] = gv.astype(bf16)
            else:
                h16 = xp.astype(np.float16)
                l16 = (xp - h16.astype(np.float64)).astype(np.float16)
                hi = h16.view(np.uint16).view(bf16)
                lo = l16.view(np.uint16).view(bf16)
                Ah = A.astype(np.float16)
                A_all[:, p * T : (p + 1) * T] = Ah.view(np.uint16).view(bf16)
                u_all[:, p] = u.astype(np.float16).view(np.uint16).view(bf16)
                g_all[0, p * T : (p + 1) * T] = (
                    gv.astype(np.float16).view(np.uint16).view(bf16)
                )
            xh[p], xl[p] = hi, lo
            GT_all[:, p * KB : (p + 1) * KB] = GT.astype(np.float32)
            scl_all[:, p] = scl.astype(np.float32)
        in_maps.append(
            {
                "x_hi": xh,
                "x_lo": xl,
                "A_all": A_all,
                "u_all": u_all,
                "g_all": g_all,
                "GT_all": GT_all,
                "scl_all": scl_all,
            }
        )
    return in_maps, order, slot_large


def kernel(tensor, gamma):
    from concourse.bass_utils import run_bass_kernel_spmd

    in_maps, order, slot_large = _make_in_maps(tensor, gamma)
    nc = _build_program(slot_large)
    res = run_bass_kernel_spmd(nc, in_maps, list(range(NCORES))).results
    y = np.empty((B * H, S, D), np.float32)
    for c in range(NCORES):
        yc = np.asarray(res[c]["y"]).reshape(PAIRS, S, D)
        for p in range(PAIRS):
            y[order[c * PAIRS + p]] = yc[p]
    return y.reshape(B, H, S, D)



# revision 2
# speedup vs baseline: 1.2666x; 1.2666x over previous
"""Discounted cumsum (y[b,h,t,d] = x[b,h,t,d] + gamma[h] * y[b,h,t-1,d]) on 8 trn2 cores.

Blocked parallel scan, pure data parallelism over the B*H=64 (b,h) pairs (8 per core).
SBUF layout per pair: [128 part = t-within-block, 32 blocks x 128 d].

Single-precision fp16 pipeline (error ~2^-11, gate is 2e-2): x cast to fp16 host-side
in scan layout (contiguous 8KB DMA lines), fp16 gamma-power coefficients, fp16 output
in scan layout; host un-permutes and upcasts. 8.4MB in + 8.4MB out per core.

Structure per pair, software-pipelined across pairs so the carry-chain latency of
pair p hides under pair p+1's PE work:
  - block sums: 32 accumulating N=128 matmuls into one [KB, D] PSUM tile; matmul j
    uses a sliding window of a [T, 2KB] constant whose only nonzero column is u
    (so column j of the window is u, rest zeros -> partition j gets r_j).
    No per-group PSUM->SBUF flat copies, no scatter DMA.
  - carries: one fp16 [KB,KB] x [KB,D] matmul (fp32 PSUM), round to fp16, gather
    [KB,D] -> [1, KB*D] on the gpsimd (SWDGE) ring so it never queues behind the
    1MiB x-loads (SP ring) or y-stores (ACT ring).
  - inject+scan: per 8-block group, rank-1 carry-injection matmul + triangular
    A-matrix scan matmul accumulate into an fp16 [T, 1024] PSUM tile (one bank);
    DVE/ACT alternate evacuating groups to the fp16 output tile.
"""

import numpy as np

B, H, S, D = 4, 16, 4096, 128
T = 128          # block length (matmul contraction dim)
KB = S // T      # 32 blocks per sequence
NG = 4           # blocks per scan matmul group (4*128 = 512 moving cols, fp32 PSUM)
G = KB // NG     # 8 groups per pair
NCORES = 8
PAIRS = (B * H) // NCORES  # 8 pair-slots per core

_nc_cache = {}


def _build_program():
    if "nc" in _nc_cache:
        return _nc_cache["nc"]

    import concourse.bass as bass
    import concourse.mybir as mybir
    from concourse.tile import TileContext

    f32 = mybir.dt.float32
    bf16 = mybir.dt.bfloat16
    fp16 = mybir.dt.float16

    nc = bass.Bass(trn_type="TRN2")

    x_d = nc.declare_dram_parameter("x16", [PAIRS, T, KB * D], fp16, isOutput=False)
    A_d = nc.declare_dram_parameter("A_all", [T, PAIRS * T], fp16, isOutput=False)
    U_d = nc.declare_dram_parameter("U_all", [T, PAIRS * 2 * KB], fp16, isOutput=False)
    g_d = nc.declare_dram_parameter("g_all", [1, PAIRS * T], fp16, isOutput=False)
    GT_d = nc.declare_dram_parameter("GT_all", [KB, PAIRS * KB], fp16, isOutput=False)
    y_d = nc.declare_dram_parameter("y", [PAIRS, T, KB * D], fp16, isOutput=True)

    with TileContext(nc) as tc:
        with (
            tc.tile_pool(name="const", bufs=1) as cpool,
            tc.tile_pool(name="xin", bufs=4) as xpool,
            tc.tile_pool(name="yout", bufs=2) as ypool,
            tc.tile_pool(name="r32", bufs=8) as r32pool,
            tc.tile_pool(name="c32", bufs=8) as c32pool,
            tc.tile_pool(name="cfl", bufs=4) as cfpool,
            tc.tile_pool(name="grp_ps", bufs=5, space="PSUM") as gp_pool,
            tc.tile_pool(name="r_ps", bufs=2, space="PSUM") as r_ps_pool,
            tc.tile_pool(name="c_ps", bufs=1, space="PSUM") as cp_pool,
        ):
            Ac = cpool.tile([T, PAIRS * T], fp16, tag="Ac")
            Uc = cpool.tile([T, PAIRS * 2 * KB], fp16, tag="Uc")
            gc = cpool.tile([1, PAIRS * T], fp16, tag="gc")
            GTc = cpool.tile([KB, PAIRS * KB], fp16, tag="GTc")
            nc.gpsimd.dma_start(out=Ac[:], in_=A_d[:])
            nc.gpsimd.dma_start(out=Uc[:], in_=U_d[:])
            nc.gpsimd.dma_start(out=gc[:], in_=g_d[:])
            nc.gpsimd.dma_start(out=GTc[:], in_=GT_d[:])

            def absorb(ap_src):
                # standalone bf16 ldweights: makes PE wait on that tile's DMA
                # lane here; the real matmuls self-load their own stationary.
                nc.tensor.ldweights(ap_src.bitcast(bf16))

            absorb(Ac[0:1, 0:1])
            absorb(Uc[0:1, 0:1])
            absorb(gc[0:1, 0:1])
            absorb(GTc[0:1, 0:1])

            state = {}

            def emit_load(p):
                Xh = xpool.tile([T, KB * D], fp16, tag="Xh")
                nc.sync.dma_start(out=Xh[:], in_=x_d[p])
                absorb(Xh[0:1, 0:1])
                return Xh

            def emit_upass_carry(p, Xh):
                # 32 accumulating matmuls -> R32ps[k, d] = r_k
                R32ps = r_ps_pool.tile([KB, D], f32, tag="R32ps")
                ub = p * 2 * KB
                for j in range(KB):
                    nc.tensor.matmul(
                        R32ps[:],
                        lhsT=Uc[:, ub + KB - j : ub + 2 * KB - j],
                        rhs=Xh[:, j * D : (j + 1) * D],
                        start=(j == 0), stop=(j == KB - 1),
                    )
                R32 = r32pool.tile([KB, D], fp16, tag="R32")
                nc.vector.tensor_copy(out=R32[:], in_=R32ps[:])

                # carries: C[k] = sum_j GT[j,k] r_j   (fp32 accumulate)
                cp = cp_pool.tile([KB, D], f32, tag="cp")
                nc.tensor.matmul(
                    cp[:], lhsT=GTc[:, p * KB : (p + 1) * KB], rhs=R32[:],
                    start=True, stop=True,
                )
                C32h = c32pool.tile([KB, D], fp16, tag="C32h")
                nc.vector.tensor_copy(out=C32h[:], in_=cp[:])
                # gather [KB part, d] -> [1, (k d)] on the SWDGE ring: never
                # queues behind the 1MiB loads/stores on the HWDGE rings.
                cfh = cfpool.tile([1, KB * D], fp16, tag="cfh")
                nc.gpsimd.dma_start(out=cfh[:], in_=C32h[:])
                absorb(cfh[0:1, 0:1])
                return cfh

            def emit_scan(p, Xh, cfh):
                Ys = ypool.tile([T, KB * D], fp16, tag="Ys")
                for g in range(G):
                    grp = gp_pool.tile([T, NG * D], f32, tag="grp")
                    sl = slice(g * NG * D, (g + 1) * NG * D)
                    nc.tensor.matmul(
                        grp[:], lhsT=gc[:, p * T : (p + 1) * T], rhs=cfh[:, sl],
                        start=True, stop=False,
                    )
                    nc.tensor.matmul(
                        grp[:], lhsT=Ac[:, p * T : (p + 1) * T], rhs=Xh[:, sl],
                        start=False, stop=True,
                    )
                    if g % 2 == 0:
                        nc.vector.tensor_copy(out=Ys[:, sl], in_=grp[:])
                    else:
                        nc.scalar.copy(out=Ys[:, sl], in_=grp[:])
                nc.scalar.dma_start(out=y_d[p], in_=Ys[:])

            prev = None
            for p in range(PAIRS):
                Xh = emit_load(p)
                cfh = emit_upass_carry(p, Xh)
                if prev is not None:
                    emit_scan(*prev)
                prev = (p, Xh, cfh)
            emit_scan(*prev)

    # Split excess per-instruction sync waits onto InstEventSemaphore carriers.
    import bass_rust

    bass_rust.generate_event_semaphores(nc)

    _nc_cache["nc"] = nc
    return nc


def _host_constants(g):
    """Per-pair gamma-power constants from float64."""
    pw = np.power(g, np.arange(S, dtype=np.float64))
    t_idx = np.arange(T)
    t_minus_s = t_idx[None, :] - t_idx[:, None]
    A = np.where(t_minus_s >= 0, pw[np.clip(t_minus_s, 0, None)], 0.0)  # [s, t]
    u = pw[127 - t_idx]
    gv = pw[t_idx + 1]
    pw128 = np.power(pw[T], np.arange(KB, dtype=np.float64))
    k_minus_j = np.arange(KB)[None, :] - 1 - np.arange(KB)[:, None]
    GT = np.where(k_minus_j >= 0, pw128[np.clip(k_minus_j, 0, None)], 0.0)  # [j, k]
    return A, u, gv, GT


def _make_in_maps(tensor, gamma):
    x = np.asarray(tensor, dtype=np.float32).reshape(B * H, S, D)
    gam = np.asarray(gamma, dtype=np.float64).reshape(H)

    # scan layout [s, (k, d)], one vectorized pass over all pairs
    x16 = np.ascontiguousarray(
        x.reshape(B * H, KB, T, D).transpose(0, 2, 1, 3)
    ).reshape(B * H, T, KB * D).astype(np.float16)

    in_maps = []
    for c in range(NCORES):
        A_all = np.zeros((T, PAIRS * T), np.float16)
        U_all = np.zeros((T, PAIRS * 2 * KB), np.float16)
        g_all = np.zeros((1, PAIRS * T), np.float16)
        GT_all = np.zeros((KB, PAIRS * KB), np.float16)
        for p in range(PAIRS):
            pid = c * PAIRS + p
            A, u, gv, GT = _host_constants(gam[pid % H])
            A_all[:, p * T : (p + 1) * T] = A.astype(np.float16)
            U_all[:, p * 2 * KB + KB] = u.astype(np.float16)
            g_all[0, p * T : (p + 1) * T] = gv.astype(np.float16)
            GT_all[:, p * KB : (p + 1) * KB] = GT.astype(np.float16)
        in_maps.append(
            {
                "x16": x16[c * PAIRS : (c + 1) * PAIRS],
                "A_all": A_all,
                "U_all": U_all,
                "g_all": g_all,
                "GT_all": GT_all,
            }
        )
    return in_maps


def _gather_output(results):
    ys = np.concatenate(
        [np.asarray(results[c]["y"]).reshape(PAIRS, T, KB * D) for c in range(NCORES)]
    )
    y = ys.reshape(B * H, T, KB, D).transpose(0, 2, 1, 3).astype(np.float32)
    return np.ascontiguousarray(y).reshape(B, H, S, D)


def kernel(tensor, gamma):
    from concourse.bass_utils import run_bass_kernel_spmd

    in_maps = _make_in_maps(tensor, gamma)
    nc = _build_program()
    res = run_bass_kernel_spmd(nc, in_maps, list(range(NCORES))).results
    return _gather_output(res)


# revision 3
# speedup vs baseline: 1.5924x; 1.2572x over previous
"""Discounted cumsum (y[b,h,t,d] = x[b,h,t,d] + gamma[h] * y[b,h,t-1,d]) on 8 trn2 cores.

Blocked parallel scan, pure data parallelism over the B*H=64 (b,h) pairs (8 per core).
SBUF layout per pair: [128 part = t-within-block, 32 blocks x 128 d].

Single-precision fp16 pipeline (error ~2^-11, gate is 2e-2): x cast to fp16 host-side
in scan layout (contiguous 8KB DMA lines), fp16 gamma-power coefficients, fp16 output
in scan layout; host un-permutes and upcasts. 8.4MB in + 8.4MB out per core.

Tricks:
  - Carry injection gamma^{s+1}*C[k,d] == A x (e_0 tensor gamma*C) because row 0 of
    the triangular scan matrix A is the gamma powers; the carries are ADDED INTO ROW
    0 of the X tile by the gather DMA itself (SWDGE accum_op=add), so the scan is
    ONE matmul per 4-block group with a single stationary A per pair.
  - Block sums go DIRECTLY to a [128, D] PSUM tile: matmul j writes r_j to
    partition 32*(j%4)+(j//4) using tile_position column group j%4 (4 quadrants run
    concurrently); the stationary is a 32-col sliding window of a [T, 40] constant
    whose only nonzero column is u. A zero-weight matmul clears the bank first, so
    every real matmul accumulates (start=False). No flat copies, no scatter DMA.
  - 2-deep software pipeline: pair p's scan is emitted after pair p+2's block sums,
    so the carry-chain latency (GT matmul -> fp16 round -> accum-gather, ~3-4us)
    hides under two pairs of PE work.
"""

import numpy as np

B, H, S, D = 4, 16, 4096, 128
T = 128          # block length (matmul contraction dim)
KB = S // T      # 32 blocks per sequence
NG = 4           # blocks per scan matmul group (4*128 = 512 moving cols, fp32 PSUM)
G = KB // NG     # 8 groups per pair
NCORES = 8
PAIRS = (B * H) // NCORES  # 8 pair-slots per core
UW = 40          # u-window const width: 8 window positions x 32 cols

_nc_cache = {}


def _scat_row(j):
    # PSUM partition that holds block j's sum (column group j%4, column j//4)
    return 32 * (j % 4) + (j // 4)


def _build_program():
    if "nc" in _nc_cache:
        return _nc_cache["nc"]

    import concourse.bass as bass
    import concourse.mybir as mybir
    from concourse.tile import TileContext

    f32 = mybir.dt.float32
    bf16 = mybir.dt.bfloat16
    fp16 = mybir.dt.float16
    ADD = mybir.AluOpType.add

    nc = bass.Bass(trn_type="TRN2")

    x_d = nc.declare_dram_parameter("x16", [PAIRS, T, KB * D], fp16, isOutput=False)
    A_d = nc.declare_dram_parameter("A_all", [T, PAIRS * T], fp16, isOutput=False)
    U_d = nc.declare_dram_parameter("U_all", [T, PAIRS * UW], fp16, isOutput=False)
    GT_d = nc.declare_dram_parameter("GT_all", [T, PAIRS * KB], fp16, isOutput=False)
    Z_d = nc.declare_dram_parameter("Z_all", [T, T], fp16, isOutput=False)
    y_d = nc.declare_dram_parameter("y", [PAIRS, T, KB * D], fp16, isOutput=True)

    with TileContext(nc) as tc:
        with (
            tc.tile_pool(name="const", bufs=1) as cpool,
            tc.tile_pool(name="xin", bufs=5) as xpool,
            tc.tile_pool(name="yout", bufs=2) as ypool,
            tc.tile_pool(name="r32", bufs=4) as r32pool,
            tc.tile_pool(name="c32", bufs=4) as c32pool,
            tc.tile_pool(name="grp_ps", bufs=4, space="PSUM") as gp_pool,
            tc.tile_pool(name="r_ps", bufs=2, space="PSUM") as r_ps_pool,
            tc.tile_pool(name="c_ps", bufs=2, space="PSUM") as cp_pool,
        ):
            # small consts early on the SP ring (ahead of the x loads), the
            # big A matrix on the ACT ring (idle until the first store).
            uc = cpool.tile([T, PAIRS * UW], fp16, tag="uc")
            GTc = cpool.tile([T, PAIRS * KB], fp16, tag="GTc")
            Zc = cpool.tile([T, T], fp16, tag="Zc")
            Ac = cpool.tile([T, PAIRS * T], fp16, tag="Ac")
            nc.sync.dma_start(out=uc[:], in_=U_d[:])
            nc.sync.dma_start(out=GTc[:], in_=GT_d[:])
            nc.sync.dma_start(out=Zc[:], in_=Z_d[:])
            nc.scalar.dma_start(out=Ac[:], in_=A_d[:])

            def absorb(ap_src):
                # standalone bf16 ldweights: makes PE wait on that tile's DMA
                # lane here; the real matmuls self-load their own stationary.
                nc.tensor.ldweights(ap_src.bitcast(bf16))

            absorb(uc[0:1, 0:1])
            absorb(GTc[0:1, 0:1])
            absorb(Zc[0:1, 0:1])
            absorb(Ac[0:1, 0:1])

            def emit_load(p):
                Xh = xpool.tile([T, KB * D], fp16, tag="Xh")
                nc.sync.dma_start(out=Xh[:], in_=x_d[p])
                absorb(Xh[0:1, 0:1])
                return Xh

            def emit_upass(p, Xh):
                # block sums straight into PSUM: r_j -> partition scat_row(j)
                R32ps = r_ps_pool.tile([T, D], f32, tag="R32ps")
                # bank-clear matmul: zero stationary, const rhs (always ready)
                nc.tensor.matmul(
                    R32ps[:], lhsT=Zc[:], rhs=Zc[:],
                    start=True, stop=False, skip_group_check=True,
                )
                ub = p * UW
                for j in range(KB):
                    q, w = j % 4, j // 4
                    nc.tensor.matmul(
                        R32ps[32 * q : 32 * q + 32, :],
                        lhsT=uc[:, ub + 8 - w : ub + UW - w],
                        rhs=Xh[:, j * D : (j + 1) * D],
                        start=False, stop=(j == KB - 1),
                        tile_position=(0, 32 * q),
                        skip_group_check=True,
                    )
                R32 = r32pool.tile([T, D], fp16, tag="R32")
                nc.vector.tensor_copy(out=R32[:], in_=R32ps[:])
                return R32

            def emit_carry(p, Xh, R32):
                # carries: gamma*C[k] = sum_j gamma*GT[j,k] r_j (GT rows are
                # host-scattered to match scat_row), then ADD into row 0 of
                # Xh during the gather (row 0 of A is the gamma powers, so
                # the scan matmul applies the injection for free).
                cp = cp_pool.tile([KB, D], f32, tag="cp")
                nc.tensor.matmul(
                    cp[:], lhsT=GTc[:, p * KB : (p + 1) * KB], rhs=R32[:],
                    start=True, stop=True,
                )
                C32h = c32pool.tile([KB, D], fp16, tag="C32h")
                nc.vector.tensor_copy(out=C32h[:], in_=cp[:])
                nc.gpsimd.dma_start(out=Xh[0:1, :], in_=C32h[:], accum_op=ADD)
                absorb(Xh[0:1, 0:1])

            def emit_scan(p, Xh):
                Ys = ypool.tile([T, KB * D], fp16, tag="Ys")
                for g in range(G):
                    grp = gp_pool.tile([T, NG * D], f32, tag="grp")
                    sl = slice(g * NG * D, (g + 1) * NG * D)
                    nc.tensor.matmul(
                        grp[:], lhsT=Ac[:, p * T : (p + 1) * T], rhs=Xh[:, sl],
                        start=True, stop=True,
                    )
                    if g % 2 == 0:
                        nc.vector.tensor_copy(out=Ys[:, sl], in_=grp[:])
                    else:
                        nc.scalar.copy(out=Ys[:, sl], in_=grp[:])
                nc.scalar.dma_start(out=y_d[p], in_=Ys[:])

            pend = []
            for p in range(PAIRS):
                Xh = emit_load(p)
                R32 = emit_upass(p, Xh)
                if len(pend) == 2:
                    emit_scan(*pend.pop(0))
                emit_carry(p, Xh, R32)
                pend.append((p, Xh))
            for pr in pend:
                emit_scan(*pr)

    # Split excess per-instruction sync waits onto InstEventSemaphore carriers.
    import bass_rust

    bass_rust.generate_event_semaphores(nc)

    _nc_cache["nc"] = nc
    return nc


def _host_constants(g):
    """Per-pair gamma-power constants from float64."""
    pw = np.power(g, np.arange(S, dtype=np.float64))
    t_idx = np.arange(T)
    t_minus_s = t_idx[None, :] - t_idx[:, None]
    A = np.where(t_minus_s >= 0, pw[np.clip(t_minus_s, 0, None)], 0.0)  # [s, t]
    u = pw[127 - t_idx]
    pw128 = np.power(pw[T], np.arange(KB, dtype=np.float64))
    k_minus_j = np.arange(KB)[None, :] - 1 - np.arange(KB)[:, None]
    # gamma * GT so the gathered value is exactly the row-0 injection term
    GT = g * np.where(k_minus_j >= 0, pw128[np.clip(k_minus_j, 0, None)], 0.0)
    return A, u, GT


def _make_in_maps(tensor, gamma):
    x = np.asarray(tensor, dtype=np.float32).reshape(B * H, S, D)
    gam = np.asarray(gamma, dtype=np.float64).reshape(H)

    # scan layout [s, (k, d)], one vectorized pass over all pairs
    x16 = np.ascontiguousarray(
        x.reshape(B * H, KB, T, D).transpose(0, 2, 1, 3)
    ).reshape(B * H, T, KB * D).astype(np.float16)

    in_maps = []
    for c in range(NCORES):
        A_all = np.zeros((T, PAIRS * T), np.float16)
        U_all = np.zeros((T, PAIRS * UW), np.float16)
        GT_all = np.zeros((T, PAIRS * KB), np.float16)
        for p in range(PAIRS):
            pid = c * PAIRS + p
            A, u, GT = _host_constants(gam[pid % H])
            A_all[:, p * T : (p + 1) * T] = A.astype(np.float16)
            U_all[:, p * UW + 8] = u.astype(np.float16)
            # scatter GT rows to the PSUM partition layout of the u-pass
            GTs = np.zeros((T, KB), np.float64)
            for j in range(KB):
                GTs[_scat_row(j)] = GT[j]
            GT_all[:, p * KB : (p + 1) * KB] = GTs.astype(np.float16)
        in_maps.append(
            {
                "x16": x16[c * PAIRS : (c + 1) * PAIRS],
                "A_all": A_all,
                "U_all": U_all,
                "GT_all": GT_all,
                "Z_all": np.zeros((T, T), np.float16),
            }
        )
    return in_maps


def _gather_output(results):
    ys = np.concatenate(
        [np.asarray(results[c]["y"]).reshape(PAIRS, T, KB * D) for c in range(NCORES)]
    )
    y = ys.reshape(B * H, T, KB, D).transpose(0, 2, 1, 3).astype(np.float32)
    return np.ascontiguousarray(y).reshape(B, H, S, D)


def kernel(tensor, gamma):
    from concourse.bass_utils import run_bass_kernel_spmd

    in_maps = _make_in_maps(tensor, gamma)
    nc = _build_program()
    res = run_bass_kernel_spmd(nc, in_maps, list(range(NCORES))).results
    return _gather_output(res)


# revision 4
# speedup vs baseline: 1.6331x; 1.0255x over previous
"""Discounted cumsum (y[b,h,t,d] = x[b,h,t,d] + gamma[h] * y[b,h,t-1,d]) on 8 trn2 cores.

Blocked parallel scan, pure data parallelism over the B*H=64 (b,h) pairs (8 per core).
SBUF layout per pair: [128 part = t-within-block, 32 blocks x 128 d].

Single-precision fp16 pipeline (error ~2^-11, gate is 2e-2): x cast to fp16 host-side
in scan layout (contiguous 8KB DMA lines), fp16 gamma-power coefficients, fp16 output
in scan layout; host un-permutes and upcasts. 8.4MB in + 8.4MB out per core.

Tricks:
  - Carry injection gamma^{s+1}*C[k,d] == A x (e_0 tensor gamma*C) because row 0 of
    the triangular scan matrix A is the gamma powers; the carries are ADDED INTO ROW
    0 of the X tile by the gather DMA itself (SWDGE accum_op=add), so the scan is
    ONE matmul per 4-block group with a single stationary A per pair.
  - Block sums go DIRECTLY to a [128, D] PSUM tile: matmul j writes r_j to
    partition 32*(j%4)+(j//4) using tile_position column group j%4 (4 quadrants run
    concurrently); the stationary is a 32-col sliding window of a [T, 40] constant
    whose only nonzero column is u. A zero-weight matmul clears the bank first, so
    every real matmul accumulates (start=False). No flat copies, no scatter DMA.
  - 2-deep software pipeline: pair p's scan is emitted after pair p+2's block sums,
    so the carry-chain latency (GT matmul -> fp16 round -> accum-gather, ~3-4us)
    hides under two pairs of PE work.
"""

import numpy as np

B, H, S, D = 4, 16, 4096, 128
T = 128          # block length (matmul contraction dim)
KB = S // T      # 32 blocks per sequence
NG = 4           # blocks per scan matmul group (4*128 = 512 moving cols, fp32 PSUM)
G = KB // NG     # 8 groups per pair
NCORES = 8
PAIRS = (B * H) // NCORES  # 8 pair-slots per core
UW = 40          # u-window const width: 8 window positions x 32 cols

_nc_cache = {}


def _scat_row(j):
    # PSUM partition that holds block j's sum (column group j%4, column j//4)
    return 32 * (j % 4) + (j // 4)


def _build_program():
    if "nc" in _nc_cache:
        return _nc_cache["nc"]

    import concourse.bass as bass
    import concourse.mybir as mybir
    from concourse.tile import TileContext

    f32 = mybir.dt.float32
    bf16 = mybir.dt.bfloat16
    fp16 = mybir.dt.float16
    ADD = mybir.AluOpType.add

    nc = bass.Bass(trn_type="TRN2")

    x_d = nc.declare_dram_parameter("x16", [PAIRS, T, KB * D], fp16, isOutput=False)
    A_d = nc.declare_dram_parameter("A_all", [T, PAIRS * T], fp16, isOutput=False)
    U_d = nc.declare_dram_parameter("U_all", [T, PAIRS * UW], fp16, isOutput=False)
    GT_d = nc.declare_dram_parameter("GT_all", [T, PAIRS * KB], fp16, isOutput=False)
    Z_d = nc.declare_dram_parameter("Z_all", [T, T], fp16, isOutput=False)
    y_d = nc.declare_dram_parameter("y", [PAIRS, T, KB * D], fp16, isOutput=True)

    with TileContext(nc) as tc:
        with (
            tc.tile_pool(name="const", bufs=1) as cpool,
            tc.tile_pool(name="xin", bufs=8) as xpool,
            tc.tile_pool(name="yout", bufs=3) as ypool,
            tc.tile_pool(name="r32", bufs=4) as r32pool,
            tc.tile_pool(name="c32", bufs=4) as c32pool,
            tc.tile_pool(name="grp_ps", bufs=5, space="PSUM") as gp_pool,
            tc.tile_pool(name="r_ps", bufs=2, space="PSUM") as r_ps_pool,
            tc.tile_pool(name="c_ps", bufs=1, space="PSUM") as cp_pool,
        ):
            # small consts early on the SP ring (ahead of the x loads), the
            # big A matrix on the ACT ring (idle until the first store).
            uc = cpool.tile([T, PAIRS * UW], fp16, tag="uc")
            GTc = cpool.tile([T, PAIRS * KB], fp16, tag="GTc")
            Zc = cpool.tile([T, T], fp16, tag="Zc")
            Ac = cpool.tile([T, PAIRS * T], fp16, tag="Ac")
            nc.sync.dma_start(out=uc[:], in_=U_d[:])
            nc.sync.dma_start(out=GTc[:], in_=GT_d[:])
            nc.sync.dma_start(out=Zc[:], in_=Z_d[:])
            nc.scalar.dma_start(out=Ac[:], in_=A_d[:])

            def absorb(ap_src):
                # standalone bf16 ldweights: makes PE wait on that tile's DMA
                # lane here; the real matmuls self-load their own stationary.
                nc.tensor.ldweights(ap_src.bitcast(bf16))

            absorb(uc[0:1, 0:1])
            absorb(GTc[0:1, 0:1])
            absorb(Zc[0:1, 0:1])
            absorb(Ac[0:1, 0:1])

            def emit_load(p):
                Xh = xpool.tile([T, KB * D], fp16, tag="Xh")
                nc.sync.dma_start(out=Xh[:], in_=x_d[p])
                absorb(Xh[0:1, 0:1])
                return Xh

            def emit_upass(p, Xh):
                # block sums straight into PSUM: r_j -> partition scat_row(j)
                R32ps = r_ps_pool.tile([T, D], f32, tag="R32ps")
                # bank-clear matmul: zero stationary, const rhs (always ready)
                nc.tensor.matmul(
                    R32ps[:], lhsT=Zc[:], rhs=Zc[:],
                    start=True, stop=False, skip_group_check=True,
                )
                ub = p * UW
                for j in range(KB):
                    q, w = j % 4, j // 4
                    nc.tensor.matmul(
                        R32ps[32 * q : 32 * q + 32, :],
                        lhsT=uc[:, ub + 8 - w : ub + UW - w],
                        rhs=Xh[:, j * D : (j + 1) * D],
                        start=False, stop=(j == KB - 1),
                        tile_position=(0, 32 * q),
                        skip_group_check=True,
                    )
                R32 = r32pool.tile([T, D], fp16, tag="R32")
                nc.vector.tensor_copy(out=R32[:], in_=R32ps[:])
                return R32

            def emit_carry(p, Xh, R32):
                # carries: gamma*C[k] = sum_j gamma*GT[j,k] r_j (GT rows are
                # host-scattered to match scat_row), then ADD into row 0 of
                # Xh during the gather (row 0 of A is the gamma powers, so
                # the scan matmul applies the injection for free).
                cp = cp_pool.tile([KB, D], f32, tag="cp")
                nc.tensor.matmul(
                    cp[:], lhsT=GTc[:, p * KB : (p + 1) * KB], rhs=R32[:],
                    start=True, stop=True,
                )
                C32h = c32pool.tile([KB, D], fp16, tag="C32h")
                nc.scalar.copy(out=C32h[:], in_=cp[:])
                nc.gpsimd.dma_start(out=Xh[0:1, :], in_=C32h[:], accum_op=ADD)
                absorb(Xh[0:1, 0:1])

            def emit_scan(p, Xh):
                Ys = ypool.tile([T, KB * D], fp16, tag="Ys")
                for g in range(G):
                    grp = gp_pool.tile([T, NG * D], f32, tag="grp")
                    sl = slice(g * NG * D, (g + 1) * NG * D)
                    nc.tensor.matmul(
                        grp[:], lhsT=Ac[:, p * T : (p + 1) * T], rhs=Xh[:, sl],
                        start=True, stop=True,
                    )
                    if g % 2 == 0:
                        nc.vector.tensor_copy(out=Ys[:, sl], in_=grp[:])
                    else:
                        nc.scalar.copy(out=Ys[:, sl], in_=grp[:])
                nc.scalar.dma_start(out=y_d[p], in_=Ys[:])

            pend = []
            for p in range(PAIRS):
                Xh = emit_load(p)
                R32 = emit_upass(p, Xh)
                if len(pend) == 2:
                    emit_scan(*pend.pop(0))
                emit_carry(p, Xh, R32)
                pend.append((p, Xh))
            for pr in pend:
                emit_scan(*pr)

    # Split excess per-instruction sync waits onto InstEventSemaphore carriers.
    import bass_rust

    bass_rust.generate_event_semaphores(nc)

    _nc_cache["nc"] = nc
    return nc


def _host_constants(g):
    """Per-pair gamma-power constants from float64."""
    pw = np.power(g, np.arange(S, dtype=np.float64))
    t_idx = np.arange(T)
    t_minus_s = t_idx[None, :] - t_idx[:, None]
    A = np.where(t_minus_s >= 0, pw[np.clip(t_minus_s, 0, None)], 0.0)  # [s, t]
    u = pw[127 - t_idx]
    pw128 = np.power(pw[T], np.arange(KB, dtype=np.float64))
    k_minus_j = np.arange(KB)[None, :] - 1 - np.arange(KB)[:, None]
    # gamma * GT so the gathered value is exactly the row-0 injection term
    GT = g * np.where(k_minus_j >= 0, pw128[np.clip(k_minus_j, 0, None)], 0.0)
    return A, u, GT


def _make_in_maps(tensor, gamma):
    x = np.asarray(tensor, dtype=np.float32).reshape(B * H, S, D)
    gam = np.asarray(gamma, dtype=np.float64).reshape(H)

    # scan layout [s, (k, d)], one vectorized pass over all pairs
    x16 = np.ascontiguousarray(
        x.reshape(B * H, KB, T, D).transpose(0, 2, 1, 3)
    ).reshape(B * H, T, KB * D).astype(np.float16)

    in_maps = []
    for c in range(NCORES):
        A_all = np.zeros((T, PAIRS * T), np.float16)
        U_all = np.zeros((T, PAIRS * UW), np.float16)
        GT_all = np.zeros((T, PAIRS * KB), np.float16)
        for p in range(PAIRS):
            pid = c * PAIRS + p
            A, u, GT = _host_constants(gam[pid % H])
            A_all[:, p * T : (p + 1) * T] = A.astype(np.float16)
            U_all[:, p * UW + 8] = u.astype(np.float16)
            # scatter GT rows to the PSUM partition layout of the u-pass
            GTs = np.zeros((T, KB), np.float64)
            for j in range(KB):
                GTs[_scat_row(j)] = GT[j]
            GT_all[:, p * KB : (p + 1) * KB] = GTs.astype(np.float16)
        in_maps.append(
            {
                "x16": x16[c * PAIRS : (c + 1) * PAIRS],
                "A_all": A_all,
                "U_all": U_all,
                "GT_all": GT_all,
                "Z_all": np.zeros((T, T), np.float16),
            }
        )
    return in_maps


def _gather_output(results):
    ys = np.concatenate(
        [np.asarray(results[c]["y"]).reshape(PAIRS, T, KB * D) for c in range(NCORES)]
    )
    y = ys.reshape(B * H, T, KB, D).transpose(0, 2, 1, 3).astype(np.float32)
    return np.ascontiguousarray(y).reshape(B, H, S, D)


def kernel(tensor, gamma):
    from concourse.bass_utils import run_bass_kernel_spmd

    in_maps = _make_in_maps(tensor, gamma)
    nc = _build_program()
    res = run_bass_kernel_spmd(nc, in_maps, list(range(NCORES))).results
    return _gather_output(res)


# revision 5
# speedup vs baseline: 1.6476x; 1.0089x over previous
"""Discounted cumsum (y[b,h,t,d] = x[b,h,t,d] + gamma[h] * y[b,h,t-1,d]) on 8 trn2 cores.

Blocked parallel scan, pure data parallelism over the B*H=64 (b,h) pairs (8 per core).
SBUF layout per pair: [128 part = t-within-block, 32 blocks x 128 d].

Single-precision fp16 pipeline (error ~2^-11, gate is 2e-2): x cast to fp16 host-side
in scan layout (contiguous 8KB DMA lines), fp16 gamma-power coefficients, fp16 output
in scan layout; host un-permutes and upcasts. 8.4MB in + 8.4MB out per core.

Tricks:
  - Carry injection gamma^{s+1}*C[k,d] == A x (e_0 tensor gamma*C) because row 0 of
    the triangular scan matrix A is the gamma powers; the carries are ADDED INTO ROW
    0 of the X tile by the gather DMA itself (SWDGE accum_op=add), so the scan is
    ONE matmul per 4-block group with a single stationary A per pair.
  - Block sums go DIRECTLY to a [128, D] PSUM tile: matmul j writes r_j to
    partition 32*(j%4)+(j//4) using tile_position column group j%4 (4 quadrants run
    concurrently); the stationary is a 32-col sliding window of a [T, 40] constant
    whose only nonzero column is u. A zero-weight matmul clears the bank first, so
    every real matmul accumulates (start=False). No flat copies, no scatter DMA.
  - 2-deep software pipeline: pair p's scan is emitted after pair p+2's block sums,
    so the carry-chain latency (GT matmul -> fp16 round -> accum-gather, ~3-4us)
    hides under two pairs of PE work.
"""

import numpy as np

B, H, S, D = 4, 16, 4096, 128
T = 128          # block length (matmul contraction dim)
KB = S // T      # 32 blocks per sequence
NG = 4           # blocks per scan matmul group (4*128 = 512 moving cols, fp32 PSUM)
G = KB // NG     # 8 groups per pair
NCORES = 8
PAIRS = (B * H) // NCORES  # 8 pair-slots per core
UW = 40          # u-window const width: 8 window positions x 32 cols

_nc_cache = {}


def _scat_row(j):
    # PSUM partition that holds block j's sum (column group j%4, column j//4)
    return 32 * (j % 4) + (j // 4)


def _build_program():
    if "nc" in _nc_cache:
        return _nc_cache["nc"]

    import concourse.bass as bass
    import concourse.mybir as mybir
    from concourse.tile import TileContext

    f32 = mybir.dt.float32
    bf16 = mybir.dt.bfloat16
    fp16 = mybir.dt.float16
    ADD = mybir.AluOpType.add

    nc = bass.Bass(trn_type="TRN2")

    x_d = nc.declare_dram_parameter("x16", [PAIRS, T, KB * D], fp16, isOutput=False)
    A_d = nc.declare_dram_parameter("A_all", [T, PAIRS * T], fp16, isOutput=False)
    U_d = nc.declare_dram_parameter("U_all", [T, PAIRS * UW], fp16, isOutput=False)
    GT_d = nc.declare_dram_parameter("GT_all", [T, PAIRS * KB], fp16, isOutput=False)
    Z_d = nc.declare_dram_parameter("Z_all", [T, T], fp16, isOutput=False)
    y_d = nc.declare_dram_parameter("y", [PAIRS, T, KB * D], fp16, isOutput=True)

    with TileContext(nc) as tc:
        with (
            tc.tile_pool(name="const", bufs=1) as cpool,
            tc.tile_pool(name="xin", bufs=8) as xpool,
            tc.tile_pool(name="yout", bufs=3) as ypool,
            tc.tile_pool(name="r32", bufs=4) as r32pool,
            tc.tile_pool(name="c32", bufs=4) as c32pool,
            tc.tile_pool(name="grp_ps", bufs=5, space="PSUM") as gp_pool,
            tc.tile_pool(name="r_ps", bufs=2, space="PSUM") as r_ps_pool,
            tc.tile_pool(name="c_ps", bufs=1, space="PSUM") as cp_pool,
        ):
            # small consts early on the SP ring (ahead of the x loads), the
            # big A matrix on the ACT ring (idle until the first store).
            uc = cpool.tile([T, PAIRS * UW], fp16, tag="uc")
            GTc = cpool.tile([T, PAIRS * KB], fp16, tag="GTc")
            Zc = cpool.tile([T, T], fp16, tag="Zc")
            Ac = cpool.tile([T, PAIRS * T], fp16, tag="Ac")
            nc.sync.dma_start(out=uc[:], in_=U_d[:])
            nc.sync.dma_start(out=GTc[:], in_=GT_d[:])
            nc.sync.dma_start(out=Zc[:], in_=Z_d[:])
            nc.scalar.dma_start(out=Ac[:], in_=A_d[:])

            def absorb(ap_src):
                # standalone bf16 ldweights: makes PE wait on that tile's DMA
                # lane here; the real matmuls self-load their own stationary.
                nc.tensor.ldweights(ap_src.bitcast(bf16))

            absorb(uc[0:1, 0:1])
            absorb(GTc[0:1, 0:1])
            absorb(Zc[0:1, 0:1])
            absorb(Ac[0:1, 0:1])

            def emit_load(p):
                Xh = xpool.tile([T, KB * D], fp16, tag="Xh")
                nc.sync.dma_start(out=Xh[:], in_=x_d[p])
                return Xh

            def emit_upass(p, Xh):
                # block sums straight into PSUM: r_j -> partition scat_row(j)
                R32ps = r_ps_pool.tile([T, D], f32, tag="R32ps")
                # bank-clear matmul: zero stationary, const rhs (always ready)
                nc.tensor.matmul(
                    R32ps[:], lhsT=Zc[:], rhs=Zc[:],
                    start=True, stop=False, skip_group_check=True,
                )
                ub = p * UW
                for j in range(KB):
                    q, w = j % 4, j // 4
                    nc.tensor.matmul(
                        R32ps[32 * q : 32 * q + 32, :],
                        lhsT=uc[:, ub + 8 - w : ub + UW - w],
                        rhs=Xh[:, j * D : (j + 1) * D],
                        start=False, stop=(j == KB - 1),
                        tile_position=(0, 32 * q),
                        skip_group_check=True,
                    )
                R32 = r32pool.tile([T, D], fp16, tag="R32")
                nc.vector.tensor_copy(out=R32[:], in_=R32ps[:])
                return R32

            def emit_carry(p, Xh, R32):
                # carries: gamma*C[k] = sum_j gamma*GT[j,k] r_j (GT rows are
                # host-scattered to match scat_row), then ADD into row 0 of
                # Xh during the gather (row 0 of A is the gamma powers, so
                # the scan matmul applies the injection for free).
                cp = cp_pool.tile([KB, D], f32, tag="cp")
                nc.tensor.matmul(
                    cp[:], lhsT=GTc[:, p * KB : (p + 1) * KB], rhs=R32[:],
                    start=True, stop=True,
                )
                C32h = c32pool.tile([KB, D], fp16, tag="C32h")
                nc.scalar.copy(out=C32h[:], in_=cp[:])
                nc.gpsimd.dma_start(out=Xh[0:1, :], in_=C32h[:], accum_op=ADD)

            def emit_scan(p, Xh, split_store=False):
                Ys = ypool.tile([T, KB * D], fp16, tag="Ys")
                half = G // 2 * NG * D
                for g in range(G):
                    grp = gp_pool.tile([T, NG * D], f32, tag="grp")
                    sl = slice(g * NG * D, (g + 1) * NG * D)
                    nc.tensor.matmul(
                        grp[:], lhsT=Ac[:, p * T : (p + 1) * T], rhs=Xh[:, sl],
                        start=True, stop=True,
                    )
                    if g % 2 == 0:
                        nc.vector.tensor_copy(out=Ys[:, sl], in_=grp[:])
                    else:
                        nc.scalar.copy(out=Ys[:, sl], in_=grp[:])
                    if split_store and g == G // 2 - 1:
                        nc.scalar.dma_start(
                            out=y_d[p][:, 0:half], in_=Ys[:, 0:half]
                        )
                if split_store:
                    nc.scalar.dma_start(out=y_d[p][:, half:], in_=Ys[:, half:])
                else:
                    nc.scalar.dma_start(out=y_d[p], in_=Ys[:])

            pend_carry = None
            pend_scan = []
            for p in range(PAIRS):
                Xh = emit_load(p)
                R32 = emit_upass(p, Xh)
                if pend_carry is not None:
                    emit_carry(*pend_carry)
                    pend_scan.append((pend_carry[0], pend_carry[1]))
                if len(pend_scan) == 2:
                    emit_scan(*pend_scan.pop(0))
                pend_carry = (p, Xh, R32)
            emit_carry(*pend_carry)
            pend_scan.append((pend_carry[0], pend_carry[1]))
            emit_scan(*pend_scan.pop(0))
            emit_scan(*pend_scan.pop(0), split_store=True)

    # Split excess per-instruction sync waits onto InstEventSemaphore carriers.
    import bass_rust

    bass_rust.generate_event_semaphores(nc)

    _nc_cache["nc"] = nc
    return nc


def _host_constants(g):
    """Per-pair gamma-power constants from float64."""
    pw = np.power(g, np.arange(S, dtype=np.float64))
    t_idx = np.arange(T)
    t_minus_s = t_idx[None, :] - t_idx[:, None]
    A = np.where(t_minus_s >= 0, pw[np.clip(t_minus_s, 0, None)], 0.0)  # [s, t]
    u = pw[127 - t_idx]
    pw128 = np.power(pw[T], np.arange(KB, dtype=np.float64))
    k_minus_j = np.arange(KB)[None, :] - 1 - np.arange(KB)[:, None]
    # gamma * GT so the gathered value is exactly the row-0 injection term
    GT = g * np.where(k_minus_j >= 0, pw128[np.clip(k_minus_j, 0, None)], 0.0)
    return A, u, GT


def _make_in_maps(tensor, gamma):
    x = np.asarray(tensor, dtype=np.float32).reshape(B * H, S, D)
    gam = np.asarray(gamma, dtype=np.float64).reshape(H)

    # scan layout [s, (k, d)], one vectorized pass over all pairs
    x16 = np.ascontiguousarray(
        x.reshape(B * H, KB, T, D).transpose(0, 2, 1, 3)
    ).reshape(B * H, T, KB * D).astype(np.float16)

    in_maps = []
    for c in range(NCORES):
        A_all = np.zeros((T, PAIRS * T), np.float16)
        U_all = np.zeros((T, PAIRS * UW), np.float16)
        GT_all = np.zeros((T, PAIRS * KB), np.float16)
        for p in range(PAIRS):
            pid = c * PAIRS + p
            A, u, GT = _host_constants(gam[pid % H])
            A_all[:, p * T : (p + 1) * T] = A.astype(np.float16)
            U_all[:, p * UW + 8] = u.astype(np.float16)
            # scatter GT rows to the PSUM partition layout of the u-pass
            GTs = np.zeros((T, KB), np.float64)
            for j in range(KB):
                GTs[_scat_row(j)] = GT[j]
            GT_all[:, p * KB : (p + 1) * KB] = GTs.astype(np.float16)
        in_maps.append(
            {
                "x16": x16[c * PAIRS : (c + 1) * PAIRS],
                "A_all": A_all,
                "U_all": U_all,
                "GT_all": GT_all,
                "Z_all": np.zeros((T, T), np.float16),
            }
        )
    return in_maps


def _gather_output(results):
    ys = np.concatenate(
        [np.asarray(results[c]["y"]).reshape(PAIRS, T, KB * D) for c in range(NCORES)]
    )
    y = ys.reshape(B * H, T, KB, D).transpose(0, 2, 1, 3).astype(np.float32)
    return np.ascontiguousarray(y).reshape(B, H, S, D)


def kernel(tensor, gamma):
    from concourse.bass_utils import run_bass_kernel_spmd

    in_maps = _make_in_maps(tensor, gamma)
    nc = _build_program()
    res = run_bass_kernel_spmd(nc, in_maps, list(range(NCORES))).results
    return _gather_output(res)


# revision 6
# speedup vs baseline: 1.6920x; 1.0269x over previous
"""Discounted cumsum (y[b,h,t,d] = x[b,h,t,d] + gamma[h] * y[b,h,t-1,d]) on 8 trn2 cores.

Blocked parallel scan, pure data parallelism over the B*H=64 (b,h) pairs (8 per core).
SBUF layout per pair: [128 part = t-within-block, 32 blocks x 128 d].

Single-precision fp16 pipeline (error ~2^-11, gate is 2e-2): x cast to fp16 host-side
in scan layout (contiguous 8KB DMA lines), fp16 gamma-power coefficients, fp16 output
in scan layout; host un-permutes and upcasts. 8.4MB in + 8.4MB out per core.

Tricks:
  - Carry injection gamma^{s+1}*C[k,d] == A x (e_0 tensor gamma*C) because row 0 of
    the triangular scan matrix A is the gamma powers; the carries are ADDED INTO ROW
    0 of the X tile by the gather DMA itself (SWDGE accum_op=add), so the scan is
    ONE matmul per 4-block group with a single stationary A per pair.
  - Block sums go DIRECTLY to a [128, D] PSUM tile: matmul j writes r_j to
    partition 32*(j%4)+(j//4) using tile_position column group j%4 (4 quadrants run
    concurrently); the stationary is a 32-col sliding window of a [T, 40] constant
    whose only nonzero column is u. A zero-weight matmul clears the bank first, so
    every real matmul accumulates (start=False). No flat copies, no scatter DMA.
  - 2-deep software pipeline: pair p's scan is emitted after pair p+2's block sums,
    so the carry-chain latency (GT matmul -> fp16 round -> accum-gather, ~3-4us)
    hides under two pairs of PE work.
"""

import numpy as np

B, H, S, D = 4, 16, 4096, 128
T = 128          # block length (matmul contraction dim)
KB = S // T      # 32 blocks per sequence
NG = 4           # blocks per scan matmul group (4*128 = 512 moving cols, fp32 PSUM)
G = KB // NG     # 8 groups per pair
NCORES = 8
PAIRS = (B * H) // NCORES  # 8 pair-slots per core
UW = 40          # u-window const width: 8 window positions x 32 cols

_nc_cache = {}


def _scat_row(j):
    # PSUM partition that holds block j's sum (column group j%4, column j//4)
    return 32 * (j % 4) + (j // 4)


def _build_program():
    if "nc" in _nc_cache:
        return _nc_cache["nc"]

    import concourse.bass as bass
    import concourse.mybir as mybir
    from concourse.tile import TileContext

    f32 = mybir.dt.float32
    bf16 = mybir.dt.bfloat16
    fp16 = mybir.dt.float16
    ADD = mybir.AluOpType.add

    nc = bass.Bass(trn_type="TRN2")

    x_d = nc.declare_dram_parameter("x16", [PAIRS, T, KB * D], fp16, isOutput=False)
    A_d = nc.declare_dram_parameter("A_all", [T, PAIRS * T], fp16, isOutput=False)
    U_d = nc.declare_dram_parameter("U_all", [T, PAIRS * UW], fp16, isOutput=False)
    GT_d = nc.declare_dram_parameter("GT_all", [T, PAIRS * KB], fp16, isOutput=False)
    Z_d = nc.declare_dram_parameter("Z_all", [T, T], fp16, isOutput=False)
    y_d = nc.declare_dram_parameter("y", [PAIRS, T, KB * D], fp16, isOutput=True)

    with TileContext(nc) as tc:
        with (
            tc.tile_pool(name="const", bufs=1) as cpool,
            tc.tile_pool(name="xin", bufs=8) as xpool,
            tc.tile_pool(name="yout", bufs=3) as ypool,
            tc.tile_pool(name="r32", bufs=4) as r32pool,
            tc.tile_pool(name="c32", bufs=4) as c32pool,
            tc.tile_pool(name="grp_ps", bufs=5, space="PSUM") as gp_pool,
            tc.tile_pool(name="r_ps", bufs=2, space="PSUM") as r_ps_pool,
            tc.tile_pool(name="c_ps", bufs=1, space="PSUM") as cp_pool,
        ):
            # small consts early on the SP ring (ahead of the x loads), the
            # big A matrix on the ACT ring (idle until the first store).
            uc = cpool.tile([T, PAIRS * UW], fp16, tag="uc")
            GTc = cpool.tile([T, PAIRS * KB], fp16, tag="GTc")
            Zc = cpool.tile([T, T], fp16, tag="Zc")
            Ac = cpool.tile([T, PAIRS * T], fp16, tag="Ac")
            nc.sync.dma_start(out=uc[:], in_=U_d[:])
            nc.sync.dma_start(out=Zc[:], in_=Z_d[:])
            nc.scalar.dma_start(out=Ac[:], in_=A_d[:])
            # first pair's load ahead of the bulkier GT const: u-pass(0)
            # starts ~3us earlier; remaining loads follow GTc.
            X0 = xpool.tile([T, KB * D], fp16, tag="Xh")
            nc.sync.dma_start(out=X0[:], in_=x_d[0])
            nc.sync.dma_start(out=GTc[:], in_=GT_d[:])

            def absorb(ap_src):
                # standalone bf16 ldweights: makes PE wait on that tile's DMA
                # lane here; the real matmuls self-load their own stationary.
                nc.tensor.ldweights(ap_src.bitcast(bf16))

            absorb(uc[0:1, 0:1])
            absorb(GTc[0:1, 0:1])
            absorb(Zc[0:1, 0:1])
            absorb(Ac[0:1, 0:1])

            def emit_load(p):
                if p == 0:
                    return X0
                Xh = xpool.tile([T, KB * D], fp16, tag="Xh")
                nc.sync.dma_start(out=Xh[:], in_=x_d[p])
                return Xh

            def emit_upass(p, Xh):
                # block sums straight into PSUM: r_j -> partition scat_row(j)
                R32ps = r_ps_pool.tile([T, D], f32, tag="R32ps")
                # bank-clear matmul: zero stationary, const rhs (always ready)
                nc.tensor.matmul(
                    R32ps[:], lhsT=Zc[:], rhs=Zc[:],
                    start=True, stop=False, skip_group_check=True,
                )
                ub = p * UW
                for j in range(KB):
                    q, w = j % 4, j // 4
                    nc.tensor.matmul(
                        R32ps[32 * q : 32 * q + 32, :],
                        lhsT=uc[:, ub + 8 - w : ub + UW - w],
                        rhs=Xh[:, j * D : (j + 1) * D],
                        start=False, stop=(j == KB - 1),
                        tile_position=(0, 32 * q),
                        skip_group_check=True,
                    )
                R32 = r32pool.tile([T, D], fp16, tag="R32")
                nc.vector.tensor_copy(out=R32[:], in_=R32ps[:])
                return R32

            def emit_carry(p, Xh, R32):
                # carries: gamma*C[k] = sum_j gamma*GT[j,k] r_j (GT rows are
                # host-scattered to match scat_row), then ADD into row 0 of
                # Xh during the gather (row 0 of A is the gamma powers, so
                # the scan matmul applies the injection for free).
                cp = cp_pool.tile([KB, D], f32, tag="cp")
                nc.tensor.matmul(
                    cp[:], lhsT=GTc[:, p * KB : (p + 1) * KB], rhs=R32[:],
                    start=True, stop=True,
                )
                C32h = c32pool.tile([KB, D], fp16, tag="C32h")
                nc.scalar.copy(out=C32h[:], in_=cp[:])
                nc.gpsimd.dma_start(out=Xh[0:1, :], in_=C32h[:], accum_op=ADD)

            def emit_scan(p, Xh, split_store=False):
                Ys = ypool.tile([T, KB * D], fp16, tag="Ys")
                half = G // 2 * NG * D
                for g in range(G):
                    grp = gp_pool.tile([T, NG * D], f32, tag="grp")
                    sl = slice(g * NG * D, (g + 1) * NG * D)
                    nc.tensor.matmul(
                        grp[:], lhsT=Ac[:, p * T : (p + 1) * T], rhs=Xh[:, sl],
                        start=True, stop=True,
                    )
                    if g % 2 == 0:
                        nc.vector.tensor_copy(out=Ys[:, sl], in_=grp[:])
                    else:
                        nc.scalar.copy(out=Ys[:, sl], in_=grp[:])
                    if split_store and g == G // 2 - 1:
                        nc.scalar.dma_start(
                            out=y_d[p][:, 0:half], in_=Ys[:, 0:half]
                        )
                if split_store:
                    nc.scalar.dma_start(out=y_d[p][:, half:], in_=Ys[:, half:])
                else:
                    nc.scalar.dma_start(out=y_d[p], in_=Ys[:])

            pend_carry = None
            pend_scan = []
            for p in range(PAIRS):
                Xh = emit_load(p)
                R32 = emit_upass(p, Xh)
                if pend_carry is not None:
                    emit_carry(*pend_carry)
                    pend_scan.append((pend_carry[0], pend_carry[1]))
                if len(pend_scan) == 2:
                    emit_scan(*pend_scan.pop(0))
                pend_carry = (p, Xh, R32)
            emit_carry(*pend_carry)
            pend_scan.append((pend_carry[0], pend_carry[1]))
            emit_scan(*pend_scan.pop(0))
            emit_scan(*pend_scan.pop(0), split_store=True)

    # Split excess per-instruction sync waits onto InstEventSemaphore carriers.
    import bass_rust

    bass_rust.generate_event_semaphores(nc)

    _nc_cache["nc"] = nc
    return nc


def _host_constants(g):
    """Per-pair gamma-power constants from float64."""
    pw = np.power(g, np.arange(S, dtype=np.float64))
    t_idx = np.arange(T)
    t_minus_s = t_idx[None, :] - t_idx[:, None]
    A = np.where(t_minus_s >= 0, pw[np.clip(t_minus_s, 0, None)], 0.0)  # [s, t]
    u = pw[127 - t_idx]
    pw128 = np.power(pw[T], np.arange(KB, dtype=np.float64))
    k_minus_j = np.arange(KB)[None, :] - 1 - np.arange(KB)[:, None]
    # gamma * GT so the gathered value is exactly the row-0 injection term
    GT = g * np.where(k_minus_j >= 0, pw128[np.clip(k_minus_j, 0, None)], 0.0)
    return A, u, GT


def _make_in_maps(tensor, gamma):
    x = np.asarray(tensor, dtype=np.float32).reshape(B * H, S, D)
    gam = np.asarray(gamma, dtype=np.float64).reshape(H)

    # scan layout [s, (k, d)], one vectorized pass over all pairs
    x16 = np.ascontiguousarray(
        x.reshape(B * H, KB, T, D).transpose(0, 2, 1, 3)
    ).reshape(B * H, T, KB * D).astype(np.float16)

    in_maps = []
    for c in range(NCORES):
        A_all = np.zeros((T, PAIRS * T), np.float16)
        U_all = np.zeros((T, PAIRS * UW), np.float16)
        GT_all = np.zeros((T, PAIRS * KB), np.float16)
        for p in range(PAIRS):
            pid = c * PAIRS + p
            A, u, GT = _host_constants(gam[pid % H])
            A_all[:, p * T : (p + 1) * T] = A.astype(np.float16)
            U_all[:, p * UW + 8] = u.astype(np.float16)
            # scatter GT rows to the PSUM partition layout of the u-pass
            GTs = np.zeros((T, KB), np.float64)
            for j in range(KB):
                GTs[_scat_row(j)] = GT[j]
            GT_all[:, p * KB : (p + 1) * KB] = GTs.astype(np.float16)
        in_maps.append(
            {
                "x16": x16[c * PAIRS : (c + 1) * PAIRS],
                "A_all": A_all,
                "U_all": U_all,
                "GT_all": GT_all,
                "Z_all": np.zeros((T, T), np.float16),
            }
        )
    return in_maps


def _gather_output(results):
    ys = np.concatenate(
        [np.asarray(results[c]["y"]).reshape(PAIRS, T, KB * D) for c in range(NCORES)]
    )
    y = ys.reshape(B * H, T, KB, D).transpose(0, 2, 1, 3).astype(np.float32)
    return np.ascontiguousarray(y).reshape(B, H, S, D)


def kernel(tensor, gamma):
    from concourse.bass_utils import run_bass_kernel_spmd

    in_maps = _make_in_maps(tensor, gamma)
    nc = _build_program()
    res = run_bass_kernel_spmd(nc, in_maps, list(range(NCORES))).results
    return _gather_output(res)
